# revision 7
# baseline (speedup 1.0000x reference)
"""BEiT-style attention block (B=64, N=197, D=768, H=12) on 8 TRN2 NeuronCores.

Data-parallel over batch: each core handles 8 batches. Per core:
  phase 1a: q,k in [dim, tok] layout (qkT = W_qk @ x.T), q pre-scaled, bias on evict
  phase 1b: v' in [tok, dim'] layout, dim' = 12 heads x (64 + ones-col), bias via
            ones-row appended to xT (contraction dim 769)
  phase 2 (per batch, head): scoresT = k.T@q in PSUM -> exp on ACT ->
            * exp(rel_pos_bias).T on DVE -> AV matmul (lhsT=v' slice) gives
            attnT rows [64, 197] + softmax-sum row -> reciprocal + DMA row
            broadcast -> DVE normalize+evict to attnT
  phase 3: proj from attnT (bias via persistent ones row), f32 out.
All matmuls bf16, fp32 accumulate. Softmax without max-subtraction (scores ~ +-3).
"""

import sys

sys.path.insert(0, "/opt/trn_rl_repo")

from contextlib import ExitStack

import numpy as np
import ml_dtypes

import concourse.bass as bass
import concourse.tile as tile
from concourse import bacc, mybir
from concourse.bass_utils import run_bass_kernel_spmd

BF16 = mybir.dt.bfloat16
F32 = mybir.dt.float32

B, N, D, H = 64, 197, 768, 12
HD = D // H  # 64
NCORES = 8
BPC = B // NCORES  # 8 batches per core
T = BPC * N  # 1576 tokens per core
KC = 6  # 768 / 128 contraction chunks
NT = 4  # token tiles for phase 1a (1576 = 4*394)
NTW = T // NT  # 394
VW = H * (HD + 1)  # 780: v' width (ones col per head)
VFT = VW // 2  # 390
MC = [(0, 128), (128, 69)]  # token m-chunks within a batch (offset, len)


def build_bass():
    nc = bacc.Bacc("TRN2")

    xT = nc.declare_dram_parameter("xT", [D + 1, T], BF16, isOutput=False)
    qkw = nc.declare_dram_parameter("qkw", [D, 2 * D], BF16, isOutput=False)
    qkb = nc.declare_dram_parameter("qkb", [128, 2 * KC], F32, isOutput=False)
    vw = nc.declare_dram_parameter("vw", [D + 1, VW], BF16, isOutput=False)
    erpb = nc.declare_dram_parameter("erpb", [H, N, N], BF16, isOutput=False)
    pw = nc.declare_dram_parameter("pw", [D + 1, D], BF16, isOutput=False)
    out = nc.declare_dram_parameter("out", [T, D], F32, isOutput=True)

    with tile.TileContext(nc) as tc, ExitStack() as ctx:
        consts = ctx.enter_context(tc.tile_pool(name="consts", bufs=1))
        big = ctx.enter_context(tc.tile_pool(name="big", bufs=1))
        work = ctx.enter_context(tc.tile_pool(name="work", bufs=3))
        attp = ctx.enter_context(tc.tile_pool(name="attp", bufs=2))
        outp = ctx.enter_context(tc.tile_pool(name="outp", bufs=3))
        pmm = ctx.enter_context(tc.tile_pool(name="pmm", bufs=3, space="PSUM"))
        pat = ctx.enter_context(tc.tile_pool(name="pat", bufs=3, space="PSUM"))
        pav = ctx.enter_context(tc.tile_pool(name="pav", bufs=2, space="PSUM"))
        rdram = ctx.enter_context(tc.tile_pool(name="rdram", bufs=4, space="DRAM"))

        # ---- constants to SBUF ----
        xT_sb = consts.tile([128, KC + 1, T], BF16)
        for c in range(KC + 1):
            rows = 128 if c < KC else 1
            nc.gpsimd.dma_start(
                out=xT_sb[:rows, c, :], in_=xT[c * 128 : c * 128 + rows, :]
            )
        qkw_sb = consts.tile([128, KC, 2 * D], BF16)
        for c in range(KC):
            nc.gpsimd.dma_start(
                out=qkw_sb[:, c, :], in_=qkw[c * 128 : (c + 1) * 128, :]
            )
        qkb_sb = consts.tile([128, 2 * KC], F32)
        nc.gpsimd.dma_start(out=qkb_sb[:], in_=qkb[:])
        vw_sb = consts.tile([128, KC + 1, VW], BF16)
        for c in range(KC + 1):
            rows = 128 if c < KC else 1
            nc.gpsimd.dma_start(
                out=vw_sb[:rows, c, :], in_=vw[c * 128 : c * 128 + rows, :]
            )
        erpb_sb = consts.tile([128, H, 2, N], BF16)
        for h in range(H):
            for mi, (mo, ml) in enumerate(MC):
                nc.gpsimd.dma_start(
                    out=erpb_sb[:ml, h, mi, :], in_=erpb[h, mo : mo + ml, :]
                )
        pw_sb = consts.tile([128, KC + 1, D], BF16)
        for c in range(KC + 1):
            rows = 128 if c < KC else 1
            nc.gpsimd.dma_start(
                out=pw_sb[:rows, c, :], in_=pw[c * 128 : c * 128 + rows, :]
            )
        ones_sb = consts.tile([1, N], BF16)
        nc.vector.memset(ones_sb[:], 1.0)

        # ---- phase 1a: qkT [2D, T] in sbuf as [128, 12, T] bf16 ----
        qk_sb = big.tile([128, 2 * KC, T], BF16)
        for oc in range(2 * KC):
            for nt in range(NT):
                ps = pmm.tile([128, NTW], F32, tag="pmm")
                for kc in range(KC):
                    nc.tensor.matmul(
                        ps[:],
                        qkw_sb[:, kc, oc * 128 : (oc + 1) * 128],
                        xT_sb[:, kc, nt * NTW : (nt + 1) * NTW],
                        start=(kc == 0),
                        stop=(kc == KC - 1),
                    )
                if oc < KC:  # q rows: add bias (scaled on host)
                    nc.scalar.activation(
                        qk_sb[:, oc, nt * NTW : (nt + 1) * NTW],
                        ps[:],
                        mybir.ActivationFunctionType.Identity,
                        bias=qkb_sb[:, oc : oc + 1],
                    )
                else:  # k rows: no bias
                    nc.scalar.activation(
                        qk_sb[:, oc, nt * NTW : (nt + 1) * NTW],
                        ps[:],
                        mybir.ActivationFunctionType.Copy,
                    )

        # ---- phase 1b: v' [tok, 780] per batch: [128, 16, 780] bf16 ----
        v_sb = big.tile([128, 2 * BPC, VW], BF16)
        for b in range(BPC):
            for mi, (mo, ml) in enumerate(MC):
                t0 = b * N + mo
                for ft in range(2):
                    ps = pmm.tile([128, VFT], F32, tag="pmm")
                    for kc in range(KC + 1):
                        rows = 128 if kc < KC else 1
                        nc.tensor.matmul(
                            ps[:ml, :],
                            xT_sb[:rows, kc, t0 : t0 + ml],
                            vw_sb[:rows, kc, ft * VFT : (ft + 1) * VFT],
                            start=(kc == 0),
                            stop=(kc == KC),
                        )
                    nc.scalar.activation(
                        v_sb[:ml, 2 * b + mi, ft * VFT : (ft + 1) * VFT],
                        ps[:ml, :],
                        mybir.ActivationFunctionType.Copy,
                    )

        # ---- phase 2: attention per (batch, head) ----
        for b in range(BPC):
            attnT = attp.tile([128, KC, N], BF16, tag="attnT")
            for h in range(H):
                oq, pq = h // 2, (h % 2) * 64
                ok = KC + oq
                ptn = []
                for mi, (mo, ml) in enumerate(MC):
                    ps_at = pat.tile([128, N], F32, tag="pat")
                    nc.tensor.matmul(
                        ps_at[:ml, :],
                        qk_sb[pq : pq + 64, ok, b * N + mo : b * N + mo + ml],
                        qk_sb[pq : pq + 64, oq, b * N : b * N + N],
                        start=True,
                        stop=True,
                    )
                    pt = work.tile([128, N], BF16, tag="pt")
                    nc.scalar.activation(
                        pt[:ml, :], ps_at[:ml, :], mybir.ActivationFunctionType.Exp
                    )
                    ptn_t = work.tile([128, N], BF16, tag="ptn")
                    nc.vector.tensor_tensor(
                        out=ptn_t[:ml, :],
                        in0=pt[:ml, :],
                        in1=erpb_sb[:ml, h, mi, :],
                        op=mybir.AluOpType.mult,
                    )
                    ptn.append(ptn_t)
                ps_o = pav.tile([128, N], F32, tag="pav")
                for mi, (mo, ml) in enumerate(MC):
                    nc.tensor.matmul(
                        ps_o[: HD + 1, :],
                        v_sb[:ml, 2 * b + mi, h * (HD + 1) : (h + 1) * (HD + 1)],
                        ptn[mi][:ml, :],
                        start=(mi == 0),
                        stop=(mi == 1),
                    )
                r_sb = work.tile([1, N], F32, tag="r")
                nc.vector.reciprocal(r_sb[:], ps_o[HD : HD + 1, :])
                rd = rdram.tile([1, N], F32, tag="rd")
                nc.gpsimd.dma_start(out=rd[:], in_=r_sb[:])
                r_exp = work.tile([64, N], F32, tag="rexp")
                r_bcast = bass.AP(
                    tensor=rd.tensor,
                    offset=rd[:].offset,
                    ap=[[0, 64]] + list(rd[:].ap[1:]),
                )
                nc.gpsimd.dma_start(out=r_exp[:], in_=r_bcast)
                nc.vector.tensor_tensor(
                    out=attnT[pq : pq + 64, ok - KC, :],
                    in0=ps_o[:HD, :],
                    in1=r_exp[:],
                    op=mybir.AluOpType.mult,
                )

            # ---- phase 3: proj for batch b ----
            for mi, (mo, ml) in enumerate(MC):
                o_sb = outp.tile([128, D], F32, tag="osb")
                for et in range(2):
                    ps = pmm.tile([128, NTW], F32, tag="pmm")
                    for kc in range(KC + 1):
                        if kc < KC:
                            lhsT = attnT[:, kc, mo : mo + ml]
                        else:
                            lhsT = ones_sb[:, mo : mo + ml]
                        nc.tensor.matmul(
                            ps[:ml, : D // 2],
                            lhsT,
                            pw_sb[: (128 if kc < KC else 1), kc,
                                  et * (D // 2) : (et + 1) * (D // 2)],
                            start=(kc == 0),
                            stop=(kc == KC),
                        )
                    nc.scalar.activation(
                        o_sb[:ml, et * (D // 2) : (et + 1) * (D // 2)],
                        ps[:ml, : D // 2],
                        mybir.ActivationFunctionType.Copy,
                    )
                nc.gpsimd.dma_start(
                    out=out[b * N + mo : b * N + mo + ml, :], in_=o_sb[:ml, :]
                )

    nc.compile()
    return nc


_NC = None


def _get_nc():
    global _NC
    if _NC is None:
        _NC = build_bass()
    return _NC


def _host_prep(x, qkv_w, q_bias, v_bias, rel_pos_bias_table, proj_w, proj_b,
               rel_pos_index):
    """Build per-core input maps (numpy, host-side sharding + layout prep)."""
    bf = ml_dtypes.bfloat16
    scale = HD ** -0.5
    Wq = qkv_w[:D] * scale
    Wk = qkv_w[D : 2 * D]
    Wv = qkv_w[2 * D :]

    qkw = np.concatenate([Wq, Wk], axis=0).T.astype(bf)  # [768, 1536]
    qkb_full = np.concatenate([q_bias * scale, np.zeros(D, np.float32)])
    qkb = qkb_full.reshape(2 * KC, 128).T.astype(np.float32).copy()  # [128, 12]

    vw = np.zeros((D + 1, VW), np.float32)
    for h in range(H):
        vw[:D, h * (HD + 1) : h * (HD + 1) + HD] = Wv[h * HD : (h + 1) * HD].T
        vw[D, h * (HD + 1) : h * (HD + 1) + HD] = v_bias[h * HD : (h + 1) * HD]
        vw[D, h * (HD + 1) + HD] = 1.0
    vw = vw.astype(bf)

    rpb = rel_pos_bias_table[rel_pos_index.reshape(-1)].reshape(N, N, H)
    erpb = np.exp(rpb.transpose(2, 1, 0).astype(np.float32)).astype(bf)  # [H, m, n]

    pw = np.concatenate([proj_w.T, proj_b[None, :]], axis=0).astype(bf)  # [769, 768]

    in_maps = []
    for c in range(NCORES):
        xc = x[c * BPC : (c + 1) * BPC].reshape(T, D).T  # [768, 1576]
        xT = np.concatenate([xc, np.ones((1, T), np.float32)], axis=0).astype(bf)
        in_maps.append(
            {"xT": np.ascontiguousarray(xT), "qkw": qkw, "qkb": qkb, "vw": vw,
             "erpb": erpb, "pw": pw}
        )
    return in_maps


def kernel(x, qkv_w, q_bias, v_bias, rel_pos_bias_table, proj_w, proj_b,
           rel_pos_index, _trace=False):
    x = np.asarray(x, np.float32)
    qkv_w = np.asarray(qkv_w, np.float32)
    q_bias = np.asarray(q_bias, np.float32)
    v_bias = np.asarray(v_bias, np.float32)
    rel_pos_bias_table = np.asarray(rel_pos_bias_table, np.float32)
    proj_w = np.asarray(proj_w, np.float32)
    proj_b = np.asarray(proj_b, np.float32)
    rel_pos_index = np.asarray(rel_pos_index)

    in_maps = _host_prep(x, qkv_w, q_bias, v_bias, rel_pos_bias_table, proj_w,
                         proj_b, rel_pos_index)
    nc = _get_nc()
    res = run_bass_kernel_spmd(nc, in_maps, core_ids=list(range(NCORES)),
                               trace=_trace)
    outs = [np.asarray(res.results[c]["out"], np.float32).reshape(BPC, N, D)
            for c in range(NCORES)]
    full = np.concatenate(outs, axis=0)
    if _trace:
        kernel._last_exec_time_ns = res.exec_time_ns
        kernel._last_result = res
    return full
